# revision 1
# baseline (speedup 1.0000x reference)
"""DRMGCN (dual-branch 5-layer GCN + channel attention + outer product) on
8 TRN2 NeuronCores.

Strategy
--------
- Graph aggregation is cast as a dense matmul against the normalized
  adjacency (random graph => no usable block sparsity): agg = A_hat @ z,
  with A_hat built on host (self-loops + symmetric normalization), padded
  to 10240 nodes, stored transposed (A_hat.T, src-major) in bf16.
- Nodes are sharded 8-way (1280/core). Each layer: local z = h @ W,
  AllGather z across cores, then aggT_shard = z_full.T-contract against
  the core's A_hat.T column slice on the tensor engine.
- h is kept TRANSPOSED [f, nodes] on-chip, which makes every matmul in
  the chain (z, agg, cnn, final) transpose-free.
- Channel attention: per-layer global max via on-chip reduce + one
  AllReduce(max); the tiny 5->25->5 MLP runs on-device; relu(att*X) ==
  att*X since X>=0 and att>0, so attention folds into the conv weights.
- Final [10000,128] @ [128,10000]: AllGather of the disease branch conv
  output (kept transposed [128, nodes]), each core emits a [1280, 10240]
  row-shard of the product.
- bf16 for all heavy matmul operands (fp32 matmul is 4x slower on PE and
  2x the DMA); fp32 accumulation in PSUM; fp32 bias/activations/output.
"""

import numpy as np
import ml_dtypes

import concourse.mybir as mybir
import concourse.tile as tile
from concourse import bacc
from concourse.bass_utils import run_bass_kernel_spmd

NC = 8          # cores
N_NODE = 10000  # real nodes per branch
NPAD = 10240    # padded (multiple of 8*128)
S = NPAD // NC  # 1280 nodes per core
P = 128
SM = S // P     # 10 m-tiles per shard
F = 256         # feature dim
FC = F // P     # 2 feature chunks
L = 5           # gcn layers
OC = 128        # conv out channels
KC = NPAD // P  # 80 contraction chunks
NT = [(0, 512), (512, 512), (1024, 256)]  # n-tiles within a 1280 shard

F32 = mybir.dt.float32
BF16 = mybir.dt.bfloat16
BF = ml_dtypes.bfloat16
AF = mybir.ActivationFunctionType
RG = [list(range(NC))]

_CACHE = {}


def _build(npad=NPAD, repeat=1, timing=False, only_layers=False,
           skip_ag=False, skip_z=False, skip_at=False,
           at_bufs=6, zk_bufs=6):
    S = npad // NC
    SM = S // P
    KC = npad // P
    NT = [(o, min(512, S - o)) for o in range(0, S, 512)]
    NPAD_ = npad
    nc = bacc.Bacc("TRN2", target_bir_lowering=False, debug=False, num_devices=NC)

    at_d, x0t_d, w_d, bt_d, cwt_d, cb_d = [], [], [], [], [], []
    fc1wt_d, fc1b_d, fc2wt_d, fc2b_d = [], [], [], []
    for br in range(2):
        at_d.append(nc.dram_tensor(f"at{br}", [KC, P, S], BF16, kind="ExternalInput"))
        x0t_d.append(nc.dram_tensor(f"x0t{br}", [FC, P, S], BF16, kind="ExternalInput"))
        w_d.append(nc.dram_tensor(f"w{br}", [L, FC, P, F], BF16, kind="ExternalInput"))
        bt_d.append(nc.dram_tensor(f"bt{br}", [L, FC, P], F32, kind="ExternalInput"))
        cwt_d.append(nc.dram_tensor(f"cwt{br}", [L, FC, P, OC], BF16, kind="ExternalInput"))
        cb_d.append(nc.dram_tensor(f"cb{br}", [P, 1], F32, kind="ExternalInput"))
        fc1wt_d.append(nc.dram_tensor(f"fc1wt{br}", [L, 5 * L], F32, kind="ExternalInput"))
        fc1b_d.append(nc.dram_tensor(f"fc1b{br}", [5 * L, 1], F32, kind="ExternalInput"))
        fc2wt_d.append(nc.dram_tensor(f"fc2wt{br}", [5 * L, L], F32, kind="ExternalInput"))
        fc2b_d.append(nc.dram_tensor(f"fc2b{br}", [L, 1], F32, kind="ExternalInput"))
    if timing:
        done_d = nc.dram_tensor("done", [P, 2 * L], F32, kind="ExternalOutput")
        out_d = None
    else:
        out_d = nc.dram_tensor("out", [S, NPAD_], BF16, kind="ExternalOutput")

    with tile.TileContext(nc) as tc:
        with (
            tc.tile_pool(name="const", bufs=1) as const,
            tc.tile_pool(name="sb", bufs=2) as sb,
            tc.tile_pool(name="zsb", bufs=2) as zsb,
            tc.tile_pool(name="zk", bufs=zk_bufs) as zkp,
            tc.tile_pool(name="atp", bufs=at_bufs) as atp,
            tc.tile_pool(name="ktp", bufs=3) as ktp,
            tc.tile_pool(name="fop", bufs=6) as fop,
            tc.tile_pool(name="psa", bufs=6, space="PSUM") as psa,
            tc.tile_pool(name="psz", bufs=2, space="PSUM") as psz,
            tc.tile_pool(name="dram", bufs=2, space="DRAM") as dram,
        ):
            mx_sb = const.tile([P, 2 * L], F32, name="mx_sb")
            nc.vector.memset(mx_sb[:], 0.0)
            ones_sb = const.tile([1, P], F32, name="ones_sb")
            nc.vector.memset(ones_sb[:], 1.0)

            w_sb, bt_sb, cwt_sb, cb_sb = [], [], [], []
            fc1wt_sb, fc1b_sb, fc2wt_sb, fc2b_sb = [], [], [], []
            for br in range(2):
                w_t = const.tile([P, L, FC, F], BF16, name=f"w_sb{br}")
                cw_t = const.tile([P, L, FC, OC], BF16, name=f"cwt_sb{br}")
                for l in range(L):
                    nc.sync.dma_start(w_t[:, l], w_d[br][l].rearrange("fc p f -> p fc f"))
                    nc.sync.dma_start(cw_t[:, l], cwt_d[br][l].rearrange("fc p o -> p fc o"))
                bt_t = const.tile([P, L, FC], F32, name=f"bt_sb{br}")
                nc.sync.dma_start(bt_t[:], bt_d[br].rearrange("l fc p -> p l fc"))
                cb_t = const.tile([P, 1], F32, name=f"cb_sb{br}")
                nc.sync.dma_start(cb_t[:], cb_d[br][:])
                f1w = const.tile([L, 5 * L], F32, name=f"fc1wt_sb{br}")
                nc.sync.dma_start(f1w[:], fc1wt_d[br][:])
                f1b = const.tile([5 * L, 1], F32, name=f"fc1b_sb{br}")
                nc.sync.dma_start(f1b[:], fc1b_d[br][:])
                f2w = const.tile([5 * L, L], F32, name=f"fc2wt_sb{br}")
                nc.sync.dma_start(f2w[:], fc2wt_d[br][:])
                f2b = const.tile([L, 1], F32, name=f"fc2b_sb{br}")
                nc.sync.dma_start(f2b[:], fc2b_d[br][:])
                w_sb.append(w_t); bt_sb.append(bt_t); cwt_sb.append(cw_t); cb_sb.append(cb_t)
                fc1wt_sb.append(f1w); fc1b_sb.append(f1b); fc2wt_sb.append(f2w); fc2b_sb.append(f2b)

            x0t_sb = []
            for br in range(2):
                x0t_t = const.tile([P, FC, S], BF16, name=f"x0t_sb{br}")
                nc.sync.dma_start(x0t_t[:], x0t_d[br].rearrange("fc p s -> p fc s"))
                x0t_sb.append(x0t_t)

            if timing:
                outbig = dram.tile([S, NPAD_], BF16, name="outbig", bufs=1)
                out_tgt = outbig
            else:
                out_tgt = out_d

            def emit():
              # ---- GCN layers; branches interleaved so one branch's
              # z/AllGather tail hides under the other's agg matmul stream ----
              ht = [[None] * L, [None] * L]
              for i in range(L):
                for br in range(2):
                    hprev = x0t_sb[br] if i == 0 else ht[br][i - 1]
                    # z_shard = h_shard @ W[i]   -> [S, F] (natural layout)
                    zf = dram.tile([NPAD_, F], BF16, name="zf", addr_space="Shared")
                    if not skip_z:
                        z_sb = zsb.tile([P, SM, F], BF16, name="z_sb")
                        for m in range(SM):
                            zp = psz.tile([P, F], F32, name="zp", tag="psz")
                            for fc in range(FC):
                                nc.tensor.matmul(
                                    zp[:],
                                    hprev[:, fc, m * P:(m + 1) * P],
                                    w_sb[br][:, i, fc, :],
                                    start=(fc == 0),
                                    stop=(fc == FC - 1),
                                )
                            nc.vector.tensor_copy(z_sb[:, m, :], zp[:])
                        zb = dram.tile([S, F], BF16, name="zb")
                        nc.sync.dma_start(zb.rearrange("(m p) f -> p m f", p=P), z_sb[:])
                        if not skip_ag:
                            nc.gpsimd.collective_compute(
                                "AllGather", mybir.AluOpType.bypass,
                                replica_groups=RG, ins=[zb.opt()], outs=[zf.opt()],
                            )
                    # aggT_shard = (A_hat @ z_full).T slice = z_full.T-contract
                    h_t = const.tile([P, FC, S], BF16, name=f"ht{br}_{i}")
                    ht[br][i] = h_t
                    aps = [[psa.tile([P, 512], F32, name="aps", tag="psa")
                            for _ in NT] for _ in range(FC)]
                    if skip_at:
                        atk0 = atp.tile([P, S], BF16, name="atk0", tag="atk0", bufs=1)
                        nc.sync.dma_start(atk0[:], at_d[br][0])
                    for k in range(KC):
                        zk = zkp.tile([P, F], BF16, name="zk")
                        nc.sync.dma_start(zk[:], zf[k * P:(k + 1) * P, :])
                        if skip_at:
                            atk = atk0
                        else:
                            atk = atp.tile([P, S], BF16, name="atk")
                            nc.sync.dma_start(atk[:], at_d[br][k])
                        for fc in range(FC):
                            for n, (off, sz) in enumerate(NT):
                                nc.tensor.matmul(
                                    aps[fc][n][:, :sz],
                                    zk[:, fc * P:(fc + 1) * P],
                                    atk[:, off:off + sz],
                                    start=(k == 0),
                                    stop=(k == KC - 1),
                                )
                    for fc in range(FC):
                        for n, (off, sz) in enumerate(NT):
                            nc.scalar.activation(
                                h_t[:, fc, off:off + sz], aps[fc][n][:, :sz],
                                AF.Relu, bias=bt_sb[br][:, i, fc:fc + 1],
                            )
                    nc.vector.reduce_max(
                        mx_sb[:, br * L + i: br * L + i + 1], h_t[:],
                        axis=mybir.AxisListType.XY,
                    )

              if not only_layers:
                # ---- attention: AllReduce(max) + tiny MLP ----
                mxb = dram.tile([P, 2 * L], F32, name="mxb")
                nc.sync.dma_start(mxb[:], mx_sb[:])
                mxr = dram.tile([P, 2 * L], F32, name="mxr", addr_space="Shared")
                nc.gpsimd.collective_compute(
                    "AllReduce", mybir.AluOpType.max,
                    replica_groups=RG, ins=[mxb.opt()], outs=[mxr.opt()],
                )
                mrow = sb.tile([1, 2 * L, P], F32, name="mrow")
                nc.sync.dma_start(mrow[:], mxr.rearrange("p i -> () i p"))
                att0 = sb.tile([1, 2 * L], F32, name="att0")
                nc.vector.reduce_max(att0[:], mrow[:], axis=mybir.AxisListType.X)
                a0d = dram.tile([1, 2 * L], F32, name="a0d")
                nc.sync.dma_start(a0d[:], att0[:])
                attf = dram.tile([1, 2 * L], F32, name="attf")
                for br in range(2):
                    a0col = sb.tile([L, 1], F32, name="a0col")
                    nc.sync.dma_start(
                        a0col[:], a0d[0:1, br * L:(br + 1) * L].rearrange("() c -> c ()")
                    )
                    p1 = psz.tile([5 * L, 1], F32, name="p1", tag="psz")
                    nc.tensor.matmul(p1[:], fc1wt_sb[br][:], a0col[:], start=True, stop=True)
                    y1 = sb.tile([5 * L, 1], F32, name="y1")
                    nc.scalar.activation(y1[:], p1[:], AF.Relu, bias=fc1b_sb[br][:])
                    p2 = psz.tile([L, 1], F32, name="p2", tag="psz")
                    nc.tensor.matmul(p2[:], fc2wt_sb[br][:], y1[:], start=True, stop=True)
                    attc = sb.tile([L, 1], F32, name="attc")
                    nc.scalar.activation(attc[:], p2[:], AF.Sigmoid, bias=fc2b_sb[br][:])
                    nc.sync.dma_start(
                        attf[0:1, br * L:(br + 1) * L].rearrange("() c -> c ()"), attc[:]
                    )
                attrow = sb.tile([1, 2 * L], F32, name="attrow")
                nc.sync.dma_start(attrow[:], attf[:])
                pb = psz.tile([P, 2 * L], F32, name="pb", tag="psz")
                nc.tensor.matmul(pb[:], ones_sb[:], attrow[:], start=True, stop=True)
                attb = sb.tile([P, 2 * L], F32, name="attb")
                nc.vector.tensor_copy(attb[:], pb[:])

                # ---- conv (attention folded into weights), transposed output ----
                oxt = []
                for br in range(2):
                    scw = const.tile([P, L, FC, OC], BF16, name=f"scw{br}")
                    for c in range(L):
                        for fc in range(FC):
                            nc.vector.tensor_scalar_mul(
                                scw[:, c, fc, :], cwt_sb[br][:, c, fc, :],
                                attb[:, br * L + c: br * L + c + 1],
                            )
                    o_t = const.tile([P, S], BF16, name=f"oxt{br}")
                    oxt.append(o_t)
                    for n, (off, sz) in enumerate(NT):
                        cps = psa.tile([P, 512], F32, name="cps", tag="psa")
                        for c in range(L):
                            for fc in range(FC):
                                nc.tensor.matmul(
                                    cps[:, :sz], scw[:, c, fc, :],
                                    ht[br][c][:, fc, off:off + sz],
                                    start=(c == 0 and fc == 0),
                                    stop=(c == L - 1 and fc == FC - 1),
                                )
                        nc.vector.tensor_scalar_add(
                            o_t[:, off:off + sz], cps[:, :sz], cb_sb[br][:]
                        )

                # ---- final: out_shard = out_x_shard @ out_y_full.T ----
                oyb = dram.tile([P, S], BF16, name="oyb")
                nc.sync.dma_start(oyb[:], oxt[1][:])
                oyf = dram.tile([NC * P, S], BF16, name="oyf", addr_space="Shared")
                nc.gpsimd.collective_compute(
                    "AllGather", mybir.AluOpType.bypass,
                    replica_groups=RG, ins=[oyb.opt()], outs=[oyf.opt()],
                )
                for r in range(NC):
                    kt = ktp.tile([P, S], BF16, name="kt")
                    nc.sync.dma_start(kt[:], oyf[r * P:(r + 1) * P, :])
                    for m in range(SM):
                        for n, (off, sz) in enumerate(NT):
                            fps = psa.tile([P, 512], F32, name="fps", tag="psa")
                            nc.tensor.matmul(
                                fps[:, :sz], oxt[0][:, m * P:(m + 1) * P],
                                kt[:, off:off + sz], start=True, stop=True,
                            )
                            fo = fop.tile([P, 512], BF16, name="fo")
                            nc.vector.tensor_copy(fo[:, :sz], fps[:, :sz])
                            nc.sync.dma_start(
                                out_tgt[m * P:(m + 1) * P, r * S + off: r * S + off + sz],
                                fo[:, :sz],
                            )

            for _ in range(repeat):
                emit()
            if timing:
                done_sb = sb.tile([P, 2 * L], F32, name="done_sb")
                nc.vector.tensor_copy(done_sb[:], mx_sb[:])
                nc.sync.dma_start(done_d[:], done_sb[:])
    nc.compile()
    return nc


def _build_at(edges, ew):
    """Dense transposed normalized adjacency A_hat.T, padded to NPAD."""
    src = np.asarray(edges[0], dtype=np.int64)
    dst = np.asarray(edges[1], dtype=np.int64)
    w = np.asarray(ew, dtype=np.float64)
    deg = np.ones(N_NODE, dtype=np.float64)  # self loops, weight 1
    np.add.at(deg, dst, w)
    dinv = 1.0 / np.sqrt(deg)
    norm = (dinv[src] * w * dinv[dst]).astype(np.float32)
    at = np.zeros((NPAD, NPAD), dtype=np.float32)
    np.add.at(at, (src, dst), norm)
    ii = np.arange(N_NODE)
    at[ii, ii] += (dinv * dinv).astype(np.float32)
    return at


def _prep_branch(x, ew, W, b, cw, cb, f1w, f1b, f2w, f2b, edges):
    at = _build_at(edges, ew)
    xp = np.zeros((NPAD, F), dtype=np.float32)
    xp[:N_NODE] = np.asarray(x, dtype=np.float32)
    x0t = np.ascontiguousarray(xp.T).astype(BF)          # [F, NPAD]
    wq = np.asarray(W, np.float32).reshape(L, FC, P, F).astype(BF)
    bt = np.asarray(b, np.float32).reshape(L, FC, P).astype(np.float32)
    cwt = np.ascontiguousarray(
        np.asarray(cw, np.float32)[:, :, :, 0].transpose(1, 2, 0)
    ).reshape(L, FC, P, OC).astype(BF)                   # [c, f, oc]
    cbq = np.asarray(cb, np.float32).reshape(P, 1)
    f1wt = np.ascontiguousarray(np.asarray(f1w, np.float32).T)  # [5,25]
    f1bq = np.asarray(f1b, np.float32).reshape(5 * L, 1)
    f2wt = np.ascontiguousarray(np.asarray(f2w, np.float32).T)  # [25,5]
    f2bq = np.asarray(f2b, np.float32).reshape(L, 1)
    return at, x0t, wq, bt, cwt, cbq, f1wt, f1bq, f2wt, f2bq


def _make_in_maps(inputs):
    br0 = _prep_branch(
        inputs["x_m"], inputs["w_m"], inputs["Wx"], inputs["bx"],
        inputs["cnnx_w"], inputs["cnnx_b"], inputs["fc1x_w"], inputs["fc1x_b"],
        inputs["fc2x_w"], inputs["fc2x_b"], inputs["edges_m"],
    )
    br1 = _prep_branch(
        inputs["x_d"], inputs["w_d"], inputs["Wy"], inputs["by"],
        inputs["cnny_w"], inputs["cnny_b"], inputs["fc1y_w"], inputs["fc1y_b"],
        inputs["fc2y_w"], inputs["fc2y_b"], inputs["edges_d"],
    )

    in_maps = []
    for k in range(NC):
        m = {}
        for br, (at, x0t, wq, bt, cwt, cbq, f1wt, f1bq, f2wt, f2bq) in enumerate(
            (br0, br1)
        ):
            sl = slice(k * S, (k + 1) * S)
            m[f"at{br}"] = np.ascontiguousarray(at[:, sl]).astype(BF).reshape(KC, P, S)
            m[f"x0t{br}"] = np.ascontiguousarray(x0t[:, sl]).reshape(FC, P, S)
            m[f"w{br}"] = wq
            m[f"bt{br}"] = bt
            m[f"cwt{br}"] = cwt
            m[f"cb{br}"] = cbq
            m[f"fc1wt{br}"] = f1wt
            m[f"fc1b{br}"] = f1bq
            m[f"fc2wt{br}"] = f2wt
            m[f"fc2b{br}"] = f2bq
        in_maps.append(m)
    return in_maps


def kernel(**inputs):
    if "nc" not in _CACHE:
        _CACHE["nc"] = _build()
    nc = _CACHE["nc"]
    in_maps = _make_in_maps(inputs)
    res = run_bass_kernel_spmd(nc, in_maps, core_ids=list(range(NC)))
    full = np.concatenate([res.results[k]["out"] for k in range(NC)], axis=0)
    return np.ascontiguousarray(full[:N_NODE, :N_NODE]).astype(np.float32)



# revision 8
# speedup vs baseline: 1.4797x; 1.4797x over previous
"""DRMGCN (dual-branch 5-layer GCN + channel attention + outer product) on
8 TRN2 NeuronCores.

Strategy (v3, mixed-precision DoubleRow)
----------------------------------------
- Graph aggregation as dense matmul against the normalized adjacency
  (random graph => no block sparsity): agg = A_hat @ z.
- Layer 0 aggregates in bf16 (fp8 noise injected at layer 0 is amplified
  ~4x through the remaining layers -- measured 5.3e-2 all-fp8 vs 1.2e-2
  with only layer 0 in bf16). Layers 1-4 aggregate in fp8-e4m3
  MatmulPerfMode.DoubleRow (157 TF/s): A_hat.T prescaled x16 in pair
  layout [KC2, P, 2, S]; z prescaled x8 (folded into W[1:]); the 1/128
  descale folds into the ReLU activation's scale.
- A one-time collective-init barrier is primed by a tiny AllGather at
  t=0 so it hides under layer-0 work instead of stalling layer 1.
- z is sharded; per-layer AllGather in pair-layout blocks so gathered
  chunks are DoubleRow-ready. Branches interleave so each branch's
  AllGather hides under the other branch's agg matmul stream.
- First RES fp8 adjacency chunks per branch stay SBUF-resident to keep
  streaming DMA under the tensor-engine time (ridge regime).
- h kept transposed [f, nodes] on-chip: z, conv and final matmuls are
  all transpose-free. Attention folds into the conv weights
  (relu(att*X) == att*X since X>=0, att>0).
- Final [10000,128] @ [128,10000]: AllGather disease-branch conv output
  (transposed [128, nodes]); each core emits a [1280, 10240] row shard;
  PSUM->bf16 copies alternate vector/scalar engines.
"""

import numpy as np
import ml_dtypes

import concourse.mybir as mybir
import concourse.tile as tile
from concourse import bacc
from concourse.bass_utils import run_bass_kernel_spmd

NC = 8          # cores
N_NODE = 10000  # real nodes per branch
NPAD = 10240    # padded (multiple of 8*256)
S = NPAD // NC  # 1280 nodes per core
P = 128
SM = S // P     # 10 m-tiles per shard
F = 256         # feature dim
FC = F // P     # 2 feature chunks
L = 5           # gcn layers
OC = 128        # conv out channels
KC = NPAD // P          # 80 bf16 contraction chunks
KC2 = NPAD // (2 * P)   # 40 DoubleRow contraction chunks
NM = NPAD // P          # 80 global m-tiles
CLM = S // (2 * P)      # 5 local paired chunks per shard
RES = 8                 # fp8 adjacency chunks resident in SBUF per branch
A_SCALE = 16.0
Z_SCALE = 8.0
DESCALE = 1.0 / (A_SCALE * Z_SCALE)
NT = [(0, 512), (512, 512), (1024, 256)]  # n-tiles within a 1280 shard

F32 = mybir.dt.float32
BF16 = mybir.dt.bfloat16
F8 = mybir.dt.float8e4
BF = ml_dtypes.bfloat16
F8NP = ml_dtypes.float8_e4m3
AF = mybir.ActivationFunctionType
PM = mybir.MatmulPerfMode.DoubleRow
RG = [list(range(NC))]

_CACHE = {}


def _build():
    nc = bacc.Bacc("TRN2", target_bir_lowering=False, debug=False, num_devices=NC)

    atb_d, at8_d, x0t_d, w_d, bt_d, cwt_d, cb_d = [], [], [], [], [], [], []
    fc1wt_d, fc1b_d, fc2wt_d, fc2b_d = [], [], [], []
    for br in range(2):
        atb_d.append(nc.dram_tensor(f"atb{br}", [KC, P, S], BF16, kind="ExternalInput"))
        at8_d.append(nc.dram_tensor(f"at8{br}", [KC2, P, 2, S], F8, kind="ExternalInput"))
        x0t_d.append(nc.dram_tensor(f"x0t{br}", [FC, P, S], BF16, kind="ExternalInput"))
        w_d.append(nc.dram_tensor(f"w{br}", [L, FC, P, F], BF16, kind="ExternalInput"))
        bt_d.append(nc.dram_tensor(f"bt{br}", [L, FC, P], F32, kind="ExternalInput"))
        cwt_d.append(nc.dram_tensor(f"cwt{br}", [L, FC, P, OC], BF16, kind="ExternalInput"))
        cb_d.append(nc.dram_tensor(f"cb{br}", [P, 1], F32, kind="ExternalInput"))
        fc1wt_d.append(nc.dram_tensor(f"fc1wt{br}", [L, 5 * L], F32, kind="ExternalInput"))
        fc1b_d.append(nc.dram_tensor(f"fc1b{br}", [5 * L, 1], F32, kind="ExternalInput"))
        fc2wt_d.append(nc.dram_tensor(f"fc2wt{br}", [5 * L, L], F32, kind="ExternalInput"))
        fc2b_d.append(nc.dram_tensor(f"fc2b{br}", [L, 1], F32, kind="ExternalInput"))
    out_d = nc.dram_tensor("out", [S, NPAD], BF16, kind="ExternalOutput")

    with tile.TileContext(nc) as tc:
        with (
            tc.tile_pool(name="const", bufs=1) as const,
            tc.tile_pool(name="sb", bufs=2) as sb,
            tc.tile_pool(name="zsb", bufs=2) as zsb,
            tc.tile_pool(name="zkp", bufs=6) as zkp,
            tc.tile_pool(name="atp", bufs=4) as atp,
            tc.tile_pool(name="ktp", bufs=4) as ktp,
            tc.tile_pool(name="fop", bufs=8) as fop,
            tc.tile_pool(name="psa", bufs=6, space="PSUM") as psa,
            tc.tile_pool(name="psz", bufs=2, space="PSUM") as psz,
            tc.tile_pool(name="dram", bufs=2, space="DRAM") as dram,
        ):
            # ---- barrier priming: route one tiny const load through an
            # AllGather so the one-time cc-init barrier runs at t=0.
            cbstage = sb.tile([P, 1], F32, name="cbstage")
            nc.sync.dma_start(cbstage[:], cb_d[0][:])
            cbb = dram.tile([P, 1], F32, name="cbb")
            nc.sync.dma_start(cbb[:], cbstage[:])
            cbf = dram.tile([NC * P, 1], F32, name="cbf", addr_space="Shared")
            nc.gpsimd.collective_compute(
                "AllGather", mybir.AluOpType.bypass,
                replica_groups=RG, ins=[cbb.opt()], outs=[cbf.opt()],
            )

            mx_sb = const.tile([P, 2 * L], F32, name="mx_sb")
            nc.vector.memset(mx_sb[:], 0.0)
            ones_sb = const.tile([1, P], F32, name="ones_sb")
            nc.vector.memset(ones_sb[:], 1.0)

            # x0 + layer-0 weights first: z0 is the head of the pipeline
            x0t_sb, w_sb = [], []
            for br in range(2):
                x0t_t = const.tile([P, FC, S], BF16, name=f"x0t_sb{br}")
                nc.sync.dma_start(x0t_t[:], x0t_d[br].rearrange("fc p s -> p fc s"))
                x0t_sb.append(x0t_t)
                w_t = const.tile([P, L, FC, F], BF16, name=f"w_sb{br}")
                nc.sync.dma_start(w_t[:, 0], w_d[br][0].rearrange("fc p f -> p fc f"))
                w_sb.append(w_t)

            # ---- z0 shard + AllGather (runs under the primed barrier) ----
            zfull = [None, None]   # current layer's gathered z (DRAM) per branch
            for br in range(2):
                z_sb = zsb.tile([P, SM, F], BF16, name="z0_sb")
                for m in range(SM):
                    zp = psz.tile([P, F], F32, name="zp", tag="psz")
                    for fc in range(FC):
                        nc.tensor.matmul(
                            zp[:],
                            x0t_sb[br][:, fc, m * P:(m + 1) * P],
                            w_sb[br][:, 0, fc, :],
                            start=(fc == 0), stop=(fc == FC - 1),
                        )
                    nc.vector.tensor_copy(z_sb[:, m, :], zp[:])
                zb = dram.tile([SM, P, F], BF16, name="zb0")
                nc.sync.dma_start(zb.rearrange("m p f -> p m f"), z_sb[:])
                zf = dram.tile([NM, P, F], BF16, name="zf0", addr_space="Shared")
                nc.gpsimd.collective_compute(
                    "AllGather", mybir.AluOpType.bypass,
                    replica_groups=RG, ins=[zb.opt()], outs=[zf.opt()],
                )
                zfull[br] = zf

            # remaining consts (needed from layer-0 relu / attention / conv)
            bt_sb, cwt_sb, fc1wt_sb, fc1b_sb, fc2wt_sb, fc2b_sb = [], [], [], [], [], []
            for br in range(2):
                for l in range(1, L):
                    nc.sync.dma_start(
                        w_sb[br][:, l], w_d[br][l].rearrange("fc p f -> p fc f")
                    )
                cw_t = const.tile([P, L, FC, OC], BF16, name=f"cwt_sb{br}")
                for l in range(L):
                    nc.sync.dma_start(cw_t[:, l], cwt_d[br][l].rearrange("fc p o -> p fc o"))
                bt_t = const.tile([P, L, FC], F32, name=f"bt_sb{br}")
                nc.sync.dma_start(bt_t[:], bt_d[br].rearrange("l fc p -> p l fc"))
                f1w = const.tile([L, 5 * L], F32, name=f"fc1wt_sb{br}")
                nc.sync.dma_start(f1w[:], fc1wt_d[br][:])
                f1b = const.tile([5 * L, 1], F32, name=f"fc1b_sb{br}")
                nc.sync.dma_start(f1b[:], fc1b_d[br][:])
                f2w = const.tile([5 * L, L], F32, name=f"fc2wt_sb{br}")
                nc.sync.dma_start(f2w[:], fc2wt_d[br][:])
                f2b = const.tile([L, 1], F32, name=f"fc2b_sb{br}")
                nc.sync.dma_start(f2b[:], fc2b_d[br][:])
                bt_sb.append(bt_t); cwt_sb.append(cw_t)
                fc1wt_sb.append(f1w); fc1b_sb.append(f1b)
                fc2wt_sb.append(f2w); fc2b_sb.append(f2b)
            cb0 = const.tile([P, 1], F32, name="cb_sb0")
            nc.sync.dma_start(cb0[:], cbf[0:P])
            cb1 = const.tile([P, 1], F32, name="cb_sb1")
            nc.sync.dma_start(cb1[:], cb_d[1][:])
            cb_sb = [cb0, cb1]

            def emit_z(br, i, h_t):
                """z_shard = h @ W[i] -> fp8 pair layout -> AllGather."""
                z_sb = zsb.tile([P, CLM, 2, F], F8, name="z_sb")
                for m in range(SM):
                    zp = psz.tile([P, F], F32, name="zp", tag="psz")
                    for fc in range(FC):
                        nc.tensor.matmul(
                            zp[:],
                            h_t[:, fc, m * P:(m + 1) * P],
                            w_sb[br][:, i, fc, :],
                            start=(fc == 0), stop=(fc == FC - 1),
                        )
                    nc.vector.tensor_copy(z_sb[:, m // 2, m % 2, :], zp[:])
                zb = dram.tile([CLM, P, 2, F], F8, name="zb")
                nc.sync.dma_start(zb.rearrange("cl p j f -> p cl j f"), z_sb[:])
                zf = dram.tile([KC2, P, 2, F], F8, name="zf", addr_space="Shared")
                nc.gpsimd.collective_compute(
                    "AllGather", mybir.AluOpType.bypass,
                    replica_groups=RG, ins=[zb.opt()], outs=[zf.opt()],
                )
                return zf

            def finish_h(br, i, h_t, aps, descale):
                for fc in range(FC):
                    for n, (off, sz) in enumerate(NT):
                        nc.scalar.activation(
                            h_t[:, fc, off:off + sz], aps[fc][n][:, :sz],
                            AF.Relu, bias=bt_sb[br][:, i, fc:fc + 1],
                            scale=descale,
                        )
                nc.vector.reduce_max(
                    mx_sb[:, br * L + i: br * L + i + 1], h_t[:],
                    axis=mybir.AxisListType.XY,
                )

            # ---- layer 0: bf16 aggregation (fp8 here dominates final error)
            ht = [[None] * L, [None] * L]
            for br in range(2):
                h_t = const.tile([P, FC, S], BF16, name=f"ht{br}_0")
                ht[br][0] = h_t
                aps = [[psa.tile([P, 512], F32, name="aps", tag="psa")
                        for _ in NT] for _ in range(FC)]
                zf = zfull[br]
                for c in range(KC):
                    zk = zkp.tile([P, F], BF16, name="zkb")
                    nc.sync.dma_start(zk[:], zf[c])
                    atk = atp.tile([P, S], BF16, name="atkb")
                    nc.sync.dma_start(atk[:], atb_d[br][c])
                    for fc in range(FC):
                        for n, (off, sz) in enumerate(NT):
                            nc.tensor.matmul(
                                aps[fc][n][:, :sz],
                                zk[:, fc * P:(fc + 1) * P],
                                atk[:, off:off + sz],
                                start=(c == 0), stop=(c == KC - 1),
                            )
                finish_h(br, 0, h_t, aps, 1.0)
                zfull[br] = emit_z(br, 1, h_t)

            # resident fp8 adjacency chunks (used by layers 1-4; loaded
            # during layer 0's tensor work)
            atres = []
            for br in range(2):
                art = const.tile([P, RES, 2, S], F8, name=f"atres{br}")
                for c in range(RES):
                    nc.sync.dma_start(art[:, c], at8_d[br][c])
                atres.append(art)

            # ---- layers 1-4: fp8 DoubleRow aggregation ----
            for i in range(1, L):
                for br in range(2):
                    h_t = const.tile([P, FC, S], BF16, name=f"ht{br}_{i}")
                    ht[br][i] = h_t
                    zf = zfull[br]
                    aps = [[psa.tile([P, 512], F32, name="aps", tag="psa")
                            for _ in NT] for _ in range(FC)]
                    for c in range(KC2):
                        zk = zkp.tile([P, 2, F], F8, name="zk")
                        nc.sync.dma_start(zk[:], zf[c])
                        if c < RES:
                            atk = atres[br][:, c]
                        else:
                            atk = atp.tile([P, 2, S], F8, name="atk")
                            nc.sync.dma_start(atk[:], at8_d[br][c])
                        for fc in range(FC):
                            for n, (off, sz) in enumerate(NT):
                                nc.tensor.matmul(
                                    aps[fc][n][:, :sz],
                                    zk[:, :, fc * P:(fc + 1) * P],
                                    atk[:, :, off:off + sz],
                                    start=(c == 0), stop=(c == KC2 - 1),
                                    perf_mode=PM,
                                )
                    finish_h(br, i, h_t, aps, DESCALE)
                    if i < L - 1:
                        zfull[br] = emit_z(br, i + 1, h_t)

            # ---- attention: AllReduce(max) + tiny MLP ----
            mxb = dram.tile([P, 2 * L], F32, name="mxb")
            nc.sync.dma_start(mxb[:], mx_sb[:])
            mxr = dram.tile([P, 2 * L], F32, name="mxr", addr_space="Shared")
            nc.gpsimd.collective_compute(
                "AllReduce", mybir.AluOpType.max,
                replica_groups=RG, ins=[mxb.opt()], outs=[mxr.opt()],
            )
            mrow = sb.tile([1, 2 * L, P], F32, name="mrow")
            nc.sync.dma_start(mrow[:], mxr.rearrange("p i -> () i p"))
            att0 = sb.tile([1, 2 * L], F32, name="att0")
            nc.vector.reduce_max(att0[:], mrow[:], axis=mybir.AxisListType.X)
            a0d = dram.tile([1, 2 * L], F32, name="a0d")
            nc.sync.dma_start(a0d[:], att0[:])
            attf = dram.tile([1, 2 * L], F32, name="attf")
            for br in range(2):
                a0col = sb.tile([L, 1], F32, name="a0col")
                nc.sync.dma_start(
                    a0col[:], a0d[0:1, br * L:(br + 1) * L].rearrange("() c -> c ()")
                )
                p1 = psz.tile([5 * L, 1], F32, name="p1", tag="psz")
                nc.tensor.matmul(p1[:], fc1wt_sb[br][:], a0col[:], start=True, stop=True)
                y1 = sb.tile([5 * L, 1], F32, name="y1")
                nc.scalar.activation(y1[:], p1[:], AF.Relu, bias=fc1b_sb[br][:])
                p2 = psz.tile([L, 1], F32, name="p2", tag="psz")
                nc.tensor.matmul(p2[:], fc2wt_sb[br][:], y1[:], start=True, stop=True)
                attc = sb.tile([L, 1], F32, name="attc")
                nc.scalar.activation(attc[:], p2[:], AF.Sigmoid, bias=fc2b_sb[br][:])
                nc.sync.dma_start(
                    attf[0:1, br * L:(br + 1) * L].rearrange("() c -> c ()"), attc[:]
                )
            attrow = sb.tile([1, 2 * L], F32, name="attrow")
            nc.sync.dma_start(attrow[:], attf[:])
            pb = psz.tile([P, 2 * L], F32, name="pb", tag="psz")
            nc.tensor.matmul(pb[:], ones_sb[:], attrow[:], start=True, stop=True)
            attb = sb.tile([P, 2 * L], F32, name="attb")
            nc.vector.tensor_copy(attb[:], pb[:])

            # ---- conv (attention folded into weights); y-branch first so
            # its AllGather overlaps the x-branch conv ----
            oxt = [None, None]
            for br in (1, 0):
                scw = const.tile([P, L, FC, OC], BF16, name=f"scw{br}")
                for c in range(L):
                    for fc in range(FC):
                        nc.vector.tensor_scalar_mul(
                            scw[:, c, fc, :], cwt_sb[br][:, c, fc, :],
                            attb[:, br * L + c: br * L + c + 1],
                        )
                o_t = const.tile([P, S], BF16, name=f"oxt{br}")
                oxt[br] = o_t
                for n, (off, sz) in enumerate(NT):
                    cps = psa.tile([P, 512], F32, name="cps", tag="psa")
                    for c in range(L):
                        for fc in range(FC):
                            nc.tensor.matmul(
                                cps[:, :sz], scw[:, c, fc, :],
                                ht[br][c][:, fc, off:off + sz],
                                start=(c == 0 and fc == 0),
                                stop=(c == L - 1 and fc == FC - 1),
                            )
                    nc.vector.tensor_scalar_add(
                        o_t[:, off:off + sz], cps[:, :sz], cb_sb[br][:]
                    )
                if br == 1:
                    oyb = dram.tile([P, S], BF16, name="oyb")
                    nc.sync.dma_start(oyb[:], o_t[:])
                    oyf = dram.tile([NC * P, S], BF16, name="oyf",
                                    addr_space="Shared")
                    nc.gpsimd.collective_compute(
                        "AllGather", mybir.AluOpType.bypass,
                        replica_groups=RG, ins=[oyb.opt()], outs=[oyf.opt()],
                    )

            # ---- final: out_shard = out_x_shard @ out_y_full.T ----
            for r in range(NC):
                kt = ktp.tile([P, S], BF16, name="kt")
                nc.sync.dma_start(kt[:], oyf[r * P:(r + 1) * P, :])
                for m in range(SM):
                    for n, (off, sz) in enumerate(NT):
                        fps = psa.tile([P, 512], F32, name="fps", tag="psa")
                        nc.tensor.matmul(
                            fps[:, :sz], oxt[0][:, m * P:(m + 1) * P],
                            kt[:, off:off + sz], start=True, stop=True,
                        )
                        fo = fop.tile([P, 512], BF16, name="fo")
                        # alternate engines for the PSUM->bf16 drain
                        if (m + n) % 2 == 0:
                            nc.vector.tensor_copy(fo[:, :sz], fps[:, :sz])
                        else:
                            nc.scalar.activation(fo[:, :sz], fps[:, :sz], AF.Copy)
                        nc.sync.dma_start(
                            out_d[m * P:(m + 1) * P, r * S + off: r * S + off + sz],
                            fo[:, :sz],
                        )
    nc.compile()
    return nc


def _build_at(edges, ew):
    """Dense transposed normalized adjacency A_hat.T, padded to NPAD."""
    src = np.asarray(edges[0], dtype=np.int64)
    dst = np.asarray(edges[1], dtype=np.int64)
    w = np.asarray(ew, dtype=np.float64)
    deg = np.ones(N_NODE, dtype=np.float64)  # self loops, weight 1
    np.add.at(deg, dst, w)
    dinv = 1.0 / np.sqrt(deg)
    norm = (dinv[src] * w * dinv[dst]).astype(np.float32)
    at = np.zeros((NPAD, NPAD), dtype=np.float32)
    np.add.at(at, (src, dst), norm)
    ii = np.arange(N_NODE)
    at[ii, ii] += (dinv * dinv).astype(np.float32)
    return at


def _prep_branch(x, ew, W, b, cw, cb, f1w, f1b, f2w, f2b, edges):
    at = _build_at(edges, ew)
    atb = at.astype(BF)                                  # bf16, layer 0
    at8 = (at * A_SCALE).astype(F8NP)                    # fp8, layers 1-4
    xp = np.zeros((NPAD, F), dtype=np.float32)
    xp[:N_NODE] = np.asarray(x, dtype=np.float32)
    x0t = np.ascontiguousarray(xp.T).astype(BF)          # [F, NPAD]
    # z prescale folded into W for fp8 layers (z0 for layer 1 uses W[1]...)
    wq = np.asarray(W, np.float32).copy()
    wq[1:] *= Z_SCALE
    wq = wq.reshape(L, FC, P, F).astype(BF)
    bt = np.asarray(b, np.float32).reshape(L, FC, P).astype(np.float32)
    cwt = np.ascontiguousarray(
        np.asarray(cw, np.float32)[:, :, :, 0].transpose(1, 2, 0)
    ).reshape(L, FC, P, OC).astype(BF)                   # [c, f, oc]
    cbq = np.asarray(cb, np.float32).reshape(P, 1)
    f1wt = np.ascontiguousarray(np.asarray(f1w, np.float32).T)  # [5,25]
    f1bq = np.asarray(f1b, np.float32).reshape(5 * L, 1)
    f2wt = np.ascontiguousarray(np.asarray(f2w, np.float32).T)  # [25,5]
    f2bq = np.asarray(f2b, np.float32).reshape(L, 1)
    return atb, at8, x0t, wq, bt, cwt, cbq, f1wt, f1bq, f2wt, f2bq


def _make_in_maps(inputs):
    br0 = _prep_branch(
        inputs["x_m"], inputs["w_m"], inputs["Wx"], inputs["bx"],
        inputs["cnnx_w"], inputs["cnnx_b"], inputs["fc1x_w"], inputs["fc1x_b"],
        inputs["fc2x_w"], inputs["fc2x_b"], inputs["edges_m"],
    )
    br1 = _prep_branch(
        inputs["x_d"], inputs["w_d"], inputs["Wy"], inputs["by"],
        inputs["cnny_w"], inputs["cnny_b"], inputs["fc1y_w"], inputs["fc1y_b"],
        inputs["fc2y_w"], inputs["fc2y_b"], inputs["edges_d"],
    )

    in_maps = []
    for k in range(NC):
        m = {}
        for br, (atb, at8, x0t, wq, bt, cwt, cbq, f1wt, f1bq, f2wt, f2bq) in enumerate(
            (br0, br1)
        ):
            sl = slice(k * S, (k + 1) * S)
            m[f"atb{br}"] = np.ascontiguousarray(atb[:, sl]).reshape(KC, P, S)
            # pair layout [c, p, j, n] = AT[c*256 + j*128 + p, n]
            m[f"at8{br}"] = np.ascontiguousarray(
                at8[:, sl].reshape(KC2, 2, P, S).transpose(0, 2, 1, 3)
            )
            m[f"x0t{br}"] = np.ascontiguousarray(x0t[:, sl]).reshape(FC, P, S)
            m[f"w{br}"] = wq
            m[f"bt{br}"] = bt
            m[f"cwt{br}"] = cwt
            m[f"cb{br}"] = cbq
            m[f"fc1wt{br}"] = f1wt
            m[f"fc1b{br}"] = f1bq
            m[f"fc2wt{br}"] = f2wt
            m[f"fc2b{br}"] = f2bq
        in_maps.append(m)
    return in_maps


def kernel(**inputs):
    if "nc" not in _CACHE:
        _CACHE["nc"] = _build()
    nc = _CACHE["nc"]
    in_maps = _make_in_maps(inputs)
    res = run_bass_kernel_spmd(nc, in_maps, core_ids=list(range(NC)))
    full = np.concatenate([res.results[k]["out"] for k in range(NC)], axis=0)
    return np.ascontiguousarray(full[:N_NODE, :N_NODE]).astype(np.float32)


# revision 18
# speedup vs baseline: 1.4964x; 1.0112x over previous
"""DRMGCN (dual-branch 5-layer GCN + channel attention + outer product) on
8 TRN2 NeuronCores.

Strategy (v3, mixed-precision DoubleRow)
----------------------------------------
- Graph aggregation as dense matmul against the normalized adjacency
  (random graph => no block sparsity): agg = A_hat @ z.
- Layer 0 aggregates in bf16 (fp8 noise injected at layer 0 is amplified
  ~4x through the remaining layers -- measured 5.3e-2 all-fp8 vs 1.2e-2
  with only layer 0 in bf16). Layers 1-4 aggregate in fp8-e4m3
  MatmulPerfMode.DoubleRow (157 TF/s): A_hat.T prescaled x16 in pair
  layout [KC2, P, 2, S]; z prescaled x8 (folded into W[1:]); the 1/128
  descale folds into the ReLU activation's scale.
- A one-time collective-init barrier is primed by a tiny AllGather at
  t=0 so it hides under layer-0 work instead of stalling layer 1.
- z is sharded; per-layer AllGather in pair-layout blocks so gathered
  chunks are DoubleRow-ready. Branches interleave so each branch's
  AllGather hides under the other branch's agg matmul stream.
- First RES fp8 adjacency chunks per branch stay SBUF-resident to keep
  streaming DMA under the tensor-engine time (ridge regime).
- h kept transposed [f, nodes] on-chip: z, conv and final matmuls are
  all transpose-free. Attention folds into the conv weights
  (relu(att*X) == att*X since X>=0, att>0).
- Final [10000,128] @ [128,10000]: AllGather disease-branch conv output
  (transposed [128, nodes]); each core emits a [1280, 10240] row shard;
  PSUM->bf16 copies alternate vector/scalar engines.
"""

import numpy as np
import ml_dtypes

import concourse.mybir as mybir
import concourse.tile as tile
from concourse import bacc
from concourse.bass_utils import run_bass_kernel_spmd

NC = 8          # cores
N_NODE = 10000  # real nodes per branch
NPAD = 10240    # padded (multiple of 8*256)
S = NPAD // NC  # 1280 nodes per core
P = 128
SM = S // P     # 10 m-tiles per shard
F = 256         # feature dim
FC = F // P     # 2 feature chunks
L = 5           # gcn layers
OC = 128        # conv out channels
KC = NPAD // P          # 80 bf16 contraction chunks
KC2 = NPAD // (2 * P)   # 40 DoubleRow contraction chunks
NM = NPAD // P          # 80 global m-tiles
CLM = S // (2 * P)      # 5 local paired chunks per shard
RES = 8                 # fp8 adjacency chunks resident in SBUF per branch
A_SCALE = 16.0
Z_SCALE = 8.0
DESCALE = 1.0 / (A_SCALE * Z_SCALE)
NT = [(0, 512), (512, 512), (1024, 256)]  # n-tiles within a 1280 shard

F32 = mybir.dt.float32
BF16 = mybir.dt.bfloat16
F8 = mybir.dt.float8e4
BF = ml_dtypes.bfloat16
F8NP = ml_dtypes.float8_e4m3
AF = mybir.ActivationFunctionType
PM = mybir.MatmulPerfMode.DoubleRow
RG = [list(range(NC))]

_CACHE = {}


def _matmul_noldw(eng, out, lhsT, rhs, start, stop, perf_mode=None):
    """nc.tensor.matmul with ldweights=False: the PE reuses the stationary
    operand loaded by a preceding nc.tensor.ldweights(), skipping the
    per-matmul LDWEIGHTS (~130 ns each, 72% overhead in DoubleRow)."""
    keep_dims = {0}
    if perf_mode in (mybir.MatmulPerfMode.DoubleRow,
                     mybir.MatmulPerfMode.DoubleRowSwInterleave):
        keep_dims.add(1)
    ifmap_ap = eng.lower_ap(rhs.opt(keep_dims), opt=False)
    weights_ap = eng.lower_ap(lhsT.opt(keep_dims), opt=False,
                              for_matmul_weights=True)
    out_ap = eng.lower_ap(out)

    def round_up(sz):
        for v in (32, 64, 128):
            if v >= sz:
                return v
        raise AssertionError(sz)

    return eng.add_instruction(
        mybir.InstMatmult(
            name=eng.bass.get_next_instruction_name(),
            replication_resolution=0,
            replication_shift_amnt=0,
            replication_num_rows=0,
            start_tensor_calc=start,
            stop_tensor_calc=stop,
            ins=[ifmap_ap, weights_ap],
            outs=[out_ap],
            perf_mode=perf_mode,
            is_transpose=None,
            ifmap_quant_offset=None,
            weights_quant_offset=None,
            bass_skip_group_check=False,
            tile_position=(lhsT.base_partition(), out.base_partition()),
            tile_size=(round_up(rhs.partition_size()),
                       round_up(out.partition_size())),
            ldweights=False,
        )
    )


def _build():
    nc = bacc.Bacc("TRN2", target_bir_lowering=False, debug=False, num_devices=NC)

    atb_d, at8_d, x0t_d, w_d, bt_d, cwt_d, cb_d = [], [], [], [], [], [], []
    fc1wt_d, fc1b_d, fc2wt_d, fc2b_d = [], [], [], []
    for br in range(2):
        atb_d.append(nc.dram_tensor(f"atb{br}", [KC, P, S], BF16, kind="ExternalInput"))
        at8_d.append(nc.dram_tensor(f"at8{br}", [KC2, P, 2, S], F8, kind="ExternalInput"))
        x0t_d.append(nc.dram_tensor(f"x0t{br}", [FC, P, S], BF16, kind="ExternalInput"))
        w_d.append(nc.dram_tensor(f"w{br}", [L, FC, P, F], BF16, kind="ExternalInput"))
        bt_d.append(nc.dram_tensor(f"bt{br}", [L, FC, P], F32, kind="ExternalInput"))
        cwt_d.append(nc.dram_tensor(f"cwt{br}", [L, FC, P, OC], BF16, kind="ExternalInput"))
        cb_d.append(nc.dram_tensor(f"cb{br}", [P, 1], F32, kind="ExternalInput"))
        fc1wt_d.append(nc.dram_tensor(f"fc1wt{br}", [L, 5 * L], F32, kind="ExternalInput"))
        fc1b_d.append(nc.dram_tensor(f"fc1b{br}", [5 * L, 1], F32, kind="ExternalInput"))
        fc2wt_d.append(nc.dram_tensor(f"fc2wt{br}", [5 * L, L], F32, kind="ExternalInput"))
        fc2b_d.append(nc.dram_tensor(f"fc2b{br}", [L, 1], F32, kind="ExternalInput"))
    out_d = nc.dram_tensor("out", [S, NPAD], BF16, kind="ExternalOutput")

    with tile.TileContext(nc) as tc:
        with (
            tc.tile_pool(name="const", bufs=1) as const,
            tc.tile_pool(name="sb", bufs=2) as sb,
            tc.tile_pool(name="zsb", bufs=2) as zsb,
            tc.tile_pool(name="zkp", bufs=6) as zkp,
            tc.tile_pool(name="atp", bufs=4) as atp,
            tc.tile_pool(name="ktp", bufs=8) as ktp,
            tc.tile_pool(name="fop", bufs=8) as fop,
            tc.tile_pool(name="psa", bufs=6, space="PSUM") as psa,
            tc.tile_pool(name="psz", bufs=2, space="PSUM") as psz,
            tc.tile_pool(name="dram", bufs=2, space="DRAM") as dram,
            tc.tile_pool(name="zbd", bufs=2, space="DRAM") as zbd,
            tc.tile_pool(name="zfd", bufs=3, space="DRAM") as zfd,
        ):
            mx_sb = const.tile([P, 2 * L], F32, name="mx_sb")
            nc.vector.memset(mx_sb[:], 0.0)
            ones_sb = const.tile([1, P], F32, name="ones_sb")
            nc.vector.memset(ones_sb[:], 1.0)

            # x0 + layer-0 weights first: z0 is the head of the pipeline
            x0t_sb, w_sb = [], []
            for br in range(2):
                x0t_t = const.tile([P, FC, S], BF16, name=f"x0t_sb{br}")
                nc.sync.dma_start(x0t_t[:], x0t_d[br].rearrange("fc p s -> p fc s"))
                x0t_sb.append(x0t_t)
                w_t = const.tile([P, L, FC, F], BF16, name=f"w_sb{br}")
                nc.sync.dma_start(w_t[:, 0], w_d[br][0].rearrange("fc p f -> p fc f"))
                w_sb.append(w_t)

            # ---- z0 shard + AllGather (runs under the primed barrier) ----
            zfull = [None, None]   # current layer's gathered z (DRAM) per branch
            for br in range(2):
                z_sb = zsb.tile([P, SM, F], BF16, name="z0_sb")
                for m in range(SM):
                    zp = psz.tile([P, F], F32, name="zp", tag="psz")
                    for fc in range(FC):
                        nc.tensor.matmul(
                            zp[:],
                            x0t_sb[br][:, fc, m * P:(m + 1) * P],
                            w_sb[br][:, 0, fc, :],
                            start=(fc == 0), stop=(fc == FC - 1),
                        )
                    nc.vector.tensor_copy(z_sb[:, m, :], zp[:])
                zb = zbd.tile([SM, P, F], BF16, name="zb0")
                nc.sync.dma_start(zb.rearrange("m p f -> p m f"), z_sb[:])
                zf = zfd.tile([NM, P, F], BF16, name="zf0", addr_space="Shared")
                nc.gpsimd.collective_compute(
                    "AllGather", mybir.AluOpType.bypass,
                    replica_groups=RG, ins=[zb.opt()], outs=[zf.opt()],
                )
                zfull[br] = zf

            # remaining consts (needed from layer-0 relu / attention / conv)
            bt_sb, cwt_sb, fc1wt_sb, fc1b_sb, fc2wt_sb, fc2b_sb = [], [], [], [], [], []
            for br in range(2):
                for l in range(1, L):
                    nc.sync.dma_start(
                        w_sb[br][:, l], w_d[br][l].rearrange("fc p f -> p fc f")
                    )
                cw_t = const.tile([P, L, FC, OC], BF16, name=f"cwt_sb{br}")
                for l in range(L):
                    nc.sync.dma_start(cw_t[:, l], cwt_d[br][l].rearrange("fc p o -> p fc o"))
                bt_t = const.tile([P, L, FC], F32, name=f"bt_sb{br}")
                nc.sync.dma_start(bt_t[:], bt_d[br].rearrange("l fc p -> p l fc"))
                f1w = const.tile([L, 5 * L], F32, name=f"fc1wt_sb{br}")
                nc.sync.dma_start(f1w[:], fc1wt_d[br][:])
                f1b = const.tile([5 * L, 1], F32, name=f"fc1b_sb{br}")
                nc.sync.dma_start(f1b[:], fc1b_d[br][:])
                f2w = const.tile([5 * L, L], F32, name=f"fc2wt_sb{br}")
                nc.sync.dma_start(f2w[:], fc2wt_d[br][:])
                f2b = const.tile([L, 1], F32, name=f"fc2b_sb{br}")
                nc.sync.dma_start(f2b[:], fc2b_d[br][:])
                bt_sb.append(bt_t); cwt_sb.append(cw_t)
                fc1wt_sb.append(f1w); fc1b_sb.append(f1b)
                fc2wt_sb.append(f2w); fc2b_sb.append(f2b)
            cb_sb = []
            for br in range(2):
                cbt = const.tile([P, 1], F32, name=f"cb_sb{br}")
                nc.sync.dma_start(cbt[:], cb_d[br][:])
                cb_sb.append(cbt)

            def emit_z(br, i, h_t):
                """z_shard = h @ W[i] -> fp8 pair layout -> AllGather."""
                z_sb = zsb.tile([P, CLM, 2, F], F8, name="z_sb")
                for m in range(SM):
                    zp = psz.tile([P, F], F32, name="zp", tag="psz")
                    for fc in range(FC):
                        nc.tensor.matmul(
                            zp[:],
                            h_t[:, fc, m * P:(m + 1) * P],
                            w_sb[br][:, i, fc, :],
                            start=(fc == 0), stop=(fc == FC - 1),
                        )
                    nc.vector.tensor_copy(z_sb[:, m // 2, m % 2, :], zp[:])
                zb = zbd.tile([CLM, P, 2, F], F8, name="zb")
                nc.sync.dma_start(zb.rearrange("cl p j f -> p cl j f"), z_sb[:])
                zf = zfd.tile([KC2, P, 2, F], F8, name="zf", addr_space="Shared")
                nc.gpsimd.collective_compute(
                    "AllGather", mybir.AluOpType.bypass,
                    replica_groups=RG, ins=[zb.opt()], outs=[zf.opt()],
                )
                return zf

            def finish_h(br, i, h_t, aps, descale):
                for fc in range(FC):
                    for n, (off, sz) in enumerate(NT):
                        nc.scalar.activation(
                            h_t[:, fc, off:off + sz], aps[fc][n][:, :sz],
                            AF.Relu, bias=bt_sb[br][:, i, fc:fc + 1],
                            scale=descale,
                        )
                nc.vector.reduce_max(
                    mx_sb[:, br * L + i: br * L + i + 1], h_t[:],
                    axis=mybir.AxisListType.XY,
                )

            # ---- layer 0: bf16 aggregation (fp8 here dominates final error)
            ht = [[None] * L, [None] * L]
            for br in range(2):
                h_t = const.tile([P, FC, S], BF16, name=f"ht{br}_0")
                ht[br][0] = h_t
                aps = [[psa.tile([P, 512], F32, name="aps", tag="psa")
                        for _ in NT] for _ in range(FC)]
                zf = zfull[br]
                for c in range(KC):
                    zk = zkp.tile([P, F], BF16, name="zkb")
                    nc.sync.dma_start(zk[:], zf[c])
                    atk = atp.tile([P, S], BF16, name="atkb")
                    nc.sync.dma_start(atk[:], atb_d[br][c])
                    for fc in range(FC):
                        nc.tensor.ldweights(zk[:, fc * P:(fc + 1) * P])
                        for n, (off, sz) in enumerate(NT):
                            _matmul_noldw(
                                nc.tensor, aps[fc][n][:, :sz],
                                zk[:, fc * P:(fc + 1) * P],
                                atk[:, off:off + sz],
                                start=(c == 0), stop=(c == KC - 1),
                            )
                finish_h(br, 0, h_t, aps, 1.0)
                zfull[br] = emit_z(br, 1, h_t)

            # resident fp8 adjacency chunks (used by layers 1-4; loaded
            # during layer 0's tensor work)
            atres = []
            for br in range(2):
                art = const.tile([P, RES, 2, S], F8, name=f"atres{br}")
                for c in range(RES):
                    nc.sync.dma_start(art[:, c], at8_d[br][c])
                atres.append(art)

            # ---- layers 1-4: fp8 DoubleRow aggregation ----
            for i in range(1, L):
                for br in range(2):
                    h_t = const.tile([P, FC, S], BF16, name=f"ht{br}_{i}")
                    ht[br][i] = h_t
                    zf = zfull[br]
                    aps = [[psa.tile([P, 512], F32, name="aps", tag="psa")
                            for _ in NT] for _ in range(FC)]
                    for c in range(KC2):
                        zk = zkp.tile([P, 2, F], F8, name="zk")
                        nc.sync.dma_start(zk[:], zf[c])
                        if c < RES:
                            atk = atres[br][:, c]
                        else:
                            atk = atp.tile([P, 2, S], F8, name="atk")
                            nc.sync.dma_start(atk[:], at8_d[br][c])
                        for fc in range(FC):
                            nc.tensor.ldweights(
                                zk[:, :, fc * P:(fc + 1) * P], perf_mode=PM
                            )
                            for n, (off, sz) in enumerate(NT):
                                _matmul_noldw(
                                    nc.tensor, aps[fc][n][:, :sz],
                                    zk[:, :, fc * P:(fc + 1) * P],
                                    atk[:, :, off:off + sz],
                                    start=(c == 0), stop=(c == KC2 - 1),
                                    perf_mode=PM,
                                )
                    finish_h(br, i, h_t, aps, DESCALE)
                    if i < L - 1:
                        zfull[br] = emit_z(br, i + 1, h_t)

            # ---- attention: AllReduce(max) + tiny MLP ----
            mxb = dram.tile([P, 2 * L], F32, name="mxb")
            nc.sync.dma_start(mxb[:], mx_sb[:])
            mxr = dram.tile([P, 2 * L], F32, name="mxr", addr_space="Shared")
            nc.gpsimd.collective_compute(
                "AllReduce", mybir.AluOpType.max,
                replica_groups=RG, ins=[mxb.opt()], outs=[mxr.opt()],
            )
            mrow = sb.tile([1, 2 * L, P], F32, name="mrow")
            nc.sync.dma_start(mrow[:], mxr.rearrange("p i -> () i p"))
            att0 = sb.tile([1, 2 * L], F32, name="att0")
            nc.vector.reduce_max(att0[:], mrow[:], axis=mybir.AxisListType.X)
            a0d = dram.tile([1, 2 * L], F32, name="a0d")
            nc.sync.dma_start(a0d[:], att0[:])
            attf = dram.tile([1, 2 * L], F32, name="attf")
            for br in range(2):
                a0col = sb.tile([L, 1], F32, name="a0col")
                nc.sync.dma_start(
                    a0col[:], a0d[0:1, br * L:(br + 1) * L].rearrange("() c -> c ()")
                )
                p1 = psz.tile([5 * L, 1], F32, name="p1", tag="psz")
                nc.tensor.matmul(p1[:], fc1wt_sb[br][:], a0col[:], start=True, stop=True)
                y1 = sb.tile([5 * L, 1], F32, name="y1")
                nc.scalar.activation(y1[:], p1[:], AF.Relu, bias=fc1b_sb[br][:])
                p2 = psz.tile([L, 1], F32, name="p2", tag="psz")
                nc.tensor.matmul(p2[:], fc2wt_sb[br][:], y1[:], start=True, stop=True)
                attc = sb.tile([L, 1], F32, name="attc")
                nc.scalar.activation(attc[:], p2[:], AF.Sigmoid, bias=fc2b_sb[br][:])
                nc.sync.dma_start(
                    attf[0:1, br * L:(br + 1) * L].rearrange("() c -> c ()"), attc[:]
                )
            attrow = sb.tile([1, 2 * L], F32, name="attrow")
            nc.sync.dma_start(attrow[:], attf[:])
            pb = psz.tile([P, 2 * L], F32, name="pb", tag="psz")
            nc.tensor.matmul(pb[:], ones_sb[:], attrow[:], start=True, stop=True)
            attb = sb.tile([P, 2 * L], F32, name="attb")
            nc.vector.tensor_copy(attb[:], pb[:])

            # ---- conv (attention folded into weights); y-branch first so
            # its AllGather overlaps the x-branch conv ----
            oxt = [None, None]
            for br in (1, 0):
                scw = const.tile([P, L, FC, OC], BF16, name=f"scw{br}")
                for c in range(L):
                    for fc in range(FC):
                        nc.vector.tensor_scalar_mul(
                            scw[:, c, fc, :], cwt_sb[br][:, c, fc, :],
                            attb[:, br * L + c: br * L + c + 1],
                        )
                o_t = const.tile([P, S], BF16, name=f"oxt{br}")
                oxt[br] = o_t
                cps = [psa.tile([P, 512], F32, name="cps", tag="psa")
                       for _ in NT]
                for c in range(L):
                    for fc in range(FC):
                        nc.tensor.ldweights(scw[:, c, fc, :])
                        for n, (off, sz) in enumerate(NT):
                            _matmul_noldw(
                                nc.tensor, cps[n][:, :sz], scw[:, c, fc, :],
                                ht[br][c][:, fc, off:off + sz],
                                start=(c == 0 and fc == 0),
                                stop=(c == L - 1 and fc == FC - 1),
                            )
                for n, (off, sz) in enumerate(NT):
                    nc.vector.tensor_scalar_add(
                        o_t[:, off:off + sz], cps[n][:, :sz], cb_sb[br][:]
                    )
                if br == 1:
                    oyb = dram.tile([P, S], BF16, name="oyb")
                    nc.sync.dma_start(oyb[:], o_t[:])
                    oyf = dram.tile([NC * P, S], BF16, name="oyf",
                                    addr_space="Shared")
                    nc.gpsimd.collective_compute(
                        "AllGather", mybir.AluOpType.bypass,
                        replica_groups=RG, ins=[oyb.opt()], outs=[oyf.opt()],
                    )

            # ---- final: out_shard = out_x_shard @ out_y_full.T ----
            # all 8 y-shards resident; one weight load per x m-tile
            kts = []
            for r in range(NC):
                kt = ktp.tile([P, S], BF16, name="kt")
                nc.sync.dma_start(kt[:], oyf[r * P:(r + 1) * P, :])
                kts.append(kt)
            for m in range(SM):
                nc.tensor.ldweights(oxt[0][:, m * P:(m + 1) * P])
                for r in range(NC):
                    for n, (off, sz) in enumerate(NT):
                        fps = psa.tile([P, 512], F32, name="fps", tag="psa")
                        _matmul_noldw(
                            nc.tensor, fps[:, :sz],
                            oxt[0][:, m * P:(m + 1) * P],
                            kts[r][:, off:off + sz], start=True, stop=True,
                        )
                        fo = fop.tile([P, 512], BF16, name="fo")
                        # alternate engines for the PSUM->bf16 drain
                        if (r + n) % 2 == 0:
                            nc.vector.tensor_copy(fo[:, :sz], fps[:, :sz])
                        else:
                            nc.scalar.activation(fo[:, :sz], fps[:, :sz], AF.Copy)
                        nc.sync.dma_start(
                            out_d[m * P:(m + 1) * P, r * S + off: r * S + off + sz],
                            fo[:, :sz],
                        )
    nc.compile()
    return nc


def _build_at(edges, ew):
    """Dense transposed normalized adjacency A_hat.T, padded to NPAD."""
    src = np.asarray(edges[0], dtype=np.int64)
    dst = np.asarray(edges[1], dtype=np.int64)
    w = np.asarray(ew, dtype=np.float64)
    deg = np.ones(N_NODE, dtype=np.float64)  # self loops, weight 1
    np.add.at(deg, dst, w)
    dinv = 1.0 / np.sqrt(deg)
    norm = (dinv[src] * w * dinv[dst]).astype(np.float32)
    at = np.zeros((NPAD, NPAD), dtype=np.float32)
    np.add.at(at, (src, dst), norm)
    ii = np.arange(N_NODE)
    at[ii, ii] += (dinv * dinv).astype(np.float32)
    return at


def _prep_branch(x, ew, W, b, cw, cb, f1w, f1b, f2w, f2b, edges):
    at = _build_at(edges, ew)
    atb = at.astype(BF)                                  # bf16, layer 0
    at8 = (at * A_SCALE).astype(F8NP)                    # fp8, layers 1-4
    xp = np.zeros((NPAD, F), dtype=np.float32)
    xp[:N_NODE] = np.asarray(x, dtype=np.float32)
    x0t = np.ascontiguousarray(xp.T).astype(BF)          # [F, NPAD]
    # z prescale folded into W for fp8 layers (z0 for layer 1 uses W[1]...)
    wq = np.asarray(W, np.float32).copy()
    wq[1:] *= Z_SCALE
    wq = wq.reshape(L, FC, P, F).astype(BF)
    bt = np.asarray(b, np.float32).reshape(L, FC, P).astype(np.float32)
    cwt = np.ascontiguousarray(
        np.asarray(cw, np.float32)[:, :, :, 0].transpose(1, 2, 0)
    ).reshape(L, FC, P, OC).astype(BF)                   # [c, f, oc]
    cbq = np.asarray(cb, np.float32).reshape(P, 1)
    f1wt = np.ascontiguousarray(np.asarray(f1w, np.float32).T)  # [5,25]
    f1bq = np.asarray(f1b, np.float32).reshape(5 * L, 1)
    f2wt = np.ascontiguousarray(np.asarray(f2w, np.float32).T)  # [25,5]
    f2bq = np.asarray(f2b, np.float32).reshape(L, 1)
    return atb, at8, x0t, wq, bt, cwt, cbq, f1wt, f1bq, f2wt, f2bq


def _make_in_maps(inputs):
    br0 = _prep_branch(
        inputs["x_m"], inputs["w_m"], inputs["Wx"], inputs["bx"],
        inputs["cnnx_w"], inputs["cnnx_b"], inputs["fc1x_w"], inputs["fc1x_b"],
        inputs["fc2x_w"], inputs["fc2x_b"], inputs["edges_m"],
    )
    br1 = _prep_branch(
        inputs["x_d"], inputs["w_d"], inputs["Wy"], inputs["by"],
        inputs["cnny_w"], inputs["cnny_b"], inputs["fc1y_w"], inputs["fc1y_b"],
        inputs["fc2y_w"], inputs["fc2y_b"], inputs["edges_d"],
    )

    in_maps = []
    for k in range(NC):
        m = {}
        for br, (atb, at8, x0t, wq, bt, cwt, cbq, f1wt, f1bq, f2wt, f2bq) in enumerate(
            (br0, br1)
        ):
            sl = slice(k * S, (k + 1) * S)
            m[f"atb{br}"] = np.ascontiguousarray(atb[:, sl]).reshape(KC, P, S)
            # pair layout [c, p, j, n] = AT[c*256 + j*128 + p, n]
            m[f"at8{br}"] = np.ascontiguousarray(
                at8[:, sl].reshape(KC2, 2, P, S).transpose(0, 2, 1, 3)
            )
            m[f"x0t{br}"] = np.ascontiguousarray(x0t[:, sl]).reshape(FC, P, S)
            m[f"w{br}"] = wq
            m[f"bt{br}"] = bt
            m[f"cwt{br}"] = cwt
            m[f"cb{br}"] = cbq
            m[f"fc1wt{br}"] = f1wt
            m[f"fc1b{br}"] = f1bq
            m[f"fc2wt{br}"] = f2wt
            m[f"fc2b{br}"] = f2bq
        in_maps.append(m)
    return in_maps


def kernel(**inputs):
    if "nc" not in _CACHE:
        _CACHE["nc"] = _build()
    nc = _CACHE["nc"]
    in_maps = _make_in_maps(inputs)
    res = run_bass_kernel_spmd(nc, in_maps, core_ids=list(range(NC)))
    full = np.concatenate([res.results[k]["out"] for k in range(NC)], axis=0)
    return np.ascontiguousarray(full[:N_NODE, :N_NODE]).astype(np.float32)
